# revision 9
# baseline (speedup 1.0000x reference)
"""ContrastiveCorrelationLoss Trainium2 kernel.

Strategy (data parallel, 8 cores x 2 batches):
  - Grid sampling = dense matmul with a host-built sparse bilinear weight
    matrix W [784, 1024] (4 nonzeros per column), applied on the PE:
    sampled[c, s] = sum_px imgT[px, c] * W[px, s].
  - Each helper's .mean() collapses to per-batch scalars via bilinear forms:
      loss = -(S1 - S2 + (allmean - shift)*S3) / (B*1024*1024)
      S1 = sum_p rC1*rA*w1raw,  S2 = sum_p (rC1*u1raw)*(rA*v1raw)/1024,
      S3 = sum_p rC1*u1raw,     allmean = sum_b sum_p rA*v1raw / (B*1024^2)
    where, per batch and helper (f2/c2 = side tensors, f1/c1 = side-1):
      u1raw[p] = sum_q relu(Gc)[p,q] * rC2[q]    (Gc = c1^T c2 raw Gram)
      v1raw[p] = sum_q Gf[p,q] * rB[q]           (Gf = f1^T f2)
      w1raw[p] = sum_q relu(Gc)[p,q]*Gf[p,q]*rC2[q]*rB[q]
    On device the Grams are computed transposed ([q, p]) so the q-indexed
    norm reciprocals rC2/rB apply as per-partition ACT scales (relu folds in
    for free since rC2 > 0), and the q-contraction is a ones-vector matmul
    accumulating across q-chunks in PSUM.
  - All matmuls in bf16 (fp32 PSUM accumulate); host combine in fp64.
"""

import os
import numpy as np
import ml_dtypes

import concourse.bacc as bacc
import concourse.mybir as mybir
import concourse.tile as tile

BF16 = mybir.dt.bfloat16
F32 = mybir.dt.float32
AF = mybir.ActivationFunctionType

B, S = 16, 32
CF, CC, H, W = 768, 512, 28, 28
PX = H * W            # 784
NS = S * S            # 1024 samples per batch
NCORES = 8
BPC = B // NCORES     # 2 batches per core
KC, KP = 7, 112       # pixel chunks: 784 = 7 * 112
MF = CF // 128        # 6 feats channel chunks
MC = CC // 128        # 4 code channel chunks
NQ = NS // 128        # 8 sample chunks
SHIFTS = (0.18, 0.12, 0.46, 0.46)


def build_bass():
    nc = bacc.Bacc(None, target_bir_lowering=False, debug=True)

    imgf = nc.dram_tensor("imgf", [BPC, 4, PX, CF], BF16, kind="ExternalInput")
    imgc = nc.dram_tensor("imgc", [BPC, 4, PX, CC], BF16, kind="ExternalInput")
    wmat = nc.dram_tensor("wmat", [BPC, 2, PX, NS], BF16, kind="ExternalInput")
    uvw = nc.dram_tensor("uvw", [BPC, 4, 3, NS], F32, kind="ExternalOutput")
    rnout = nc.dram_tensor("rn", [BPC, 4, 2, 128, NQ], F32, kind="ExternalOutput")

    with tile.TileContext(nc) as tc:
        with (
            tc.tile_pool(name="consts", bufs=1) as consts,
            tc.tile_pool(name="io", bufs=2) as io,
            tc.tile_pool(name="ten", bufs=2) as ten,
            tc.tile_pool(name="work", bufs=4) as work,
            tc.tile_pool(name="rpool", bufs=4) as rpool,
            tc.tile_pool(name="dscratch", bufs=4, space="DRAM") as dscratch,
            tc.tile_pool(name="pmm", bufs=4, space="PSUM") as pmm,
            tc.tile_pool(name="pacc", bufs=4, space="PSUM") as pacc,
        ):
            ones = consts.tile([128, 1], BF16)
            nc.vector.memset(ones, 1.0)

            def sample(img_t, Mch, w_t, tag):
                """sampled[c, s] tiles [128, Mch, 1024] + sumsq accumulators."""
                T = ten.tile([128, Mch, NS], BF16, tag=tag)
                accs = [pacc.tile([1, 512], F32, tag="acc", name=f"acc{_h}") for _h in range(2)]
                def emit_norm_ones(t2, mc):
                    for h in range(2):
                        nc.tensor.matmul(
                            accs[h][0:1, :],
                            ones[:, :],
                            t2[:, h * 512:(h + 1) * 512],
                            start=(mc == 0),
                            stop=(mc == Mch - 1),
                        )

                pend = None
                for mc in range(Mch):
                    for h in range(2):
                        ps = pmm.tile([128, 512], F32, tag="mm")
                        for kc in range(KC):
                            nc.tensor.matmul(
                                ps,
                                img_t[:, kc, mc * 128:(mc + 1) * 128],
                                w_t[:, kc, h * 512:(h + 1) * 512],
                                start=(kc == 0),
                                stop=(kc == KC - 1),
                            )
                        nc.scalar.activation(
                            T[:, mc, h * 512:(h + 1) * 512], ps, AF.Copy
                        )
                    t2 = work.tile([128, NS], BF16, tag="t2")
                    nc.vector.tensor_mul(t2, T[:, mc, :], T[:, mc, :])
                    if pend is not None:
                        emit_norm_ones(*pend)
                    pend = (t2, mc)
                emit_norm_ones(*pend)
                return T, accs

            def norms(accs, lb, side, ti):
                """1/||T_s|| as [128, 8] tile (element [j, k] = r[k*128+j])."""
                ssq = work.tile([1, NS], F32, tag="ssq")
                nc.scalar.activation(ssq[:, 0:512], accs[0][0:1, :], AF.Copy)
                nc.scalar.activation(ssq[:, 512:1024], accs[1][0:1, :], AF.Copy)
                d = dscratch.tile([128, NQ], F32, tag="dssq")
                # write flat q order (k outer, j inner) into d[j, k]
                nc.sync.dma_start(out=d[:].rearrange("j k -> k j"), in_=ssq[:])
                rraw = rpool.tile([128, NQ], F32, tag="rraw")
                nc.sync.dma_start(out=rraw, in_=d[:])
                rinv = rpool.tile([128, NQ], F32, tag="rinv")
                nc.vector.reciprocal(rinv, rraw)
                rt = rpool.tile([128, NQ], F32, tag="rt")
                nc.scalar.activation(rt, rinv, AF.Sqrt)
                nc.sync.dma_start(out=rnout[lb, side, ti], in_=rt)
                return rt

            def helper(F0, C0, Fs, Cs, rtf, rtc, lb, hidx):
                accs = [pacc.tile([65, 512], F32, tag="acc", name=f"acc{_h}") for _h in range(2)]

                def emit_acc_ones(R, G2, P, qk, h):
                    st, sp = (qk == 0), (qk == NQ - 1)
                    acc = accs[h]
                    nc.tensor.matmul(acc[0:1, :], ones[:, :], R, start=st, stop=sp)
                    nc.tensor.matmul(acc[32:33, :], ones[:, :], G2, start=st, stop=sp)
                    nc.tensor.matmul(acc[64:65, :], ones[:, :], P, start=st, stop=sp)

                pend = [None]
                for qk in range(NQ):
                    for h in range(2):
                        gc = pmm.tile([128, 512], F32, tag="mm")
                        for kc in range(MC):
                            nc.tensor.matmul(
                                gc,
                                Cs[:, kc, qk * 128:(qk + 1) * 128],
                                C0[:, kc, h * 512:(h + 1) * 512],
                                start=(kc == 0),
                                stop=(kc == MC - 1),
                            )
                        R = work.tile([128, 512], BF16, tag="R")
                        nc.scalar.activation(
                            R, gc, AF.Relu, scale=rtc[:, qk:qk + 1]
                        )
                        gf = pmm.tile([128, 512], F32, tag="mm")
                        for kc in range(MF):
                            nc.tensor.matmul(
                                gf,
                                Fs[:, kc, qk * 128:(qk + 1) * 128],
                                F0[:, kc, h * 512:(h + 1) * 512],
                                start=(kc == 0),
                                stop=(kc == MF - 1),
                            )
                        G2 = work.tile([128, 512], BF16, tag="G2")
                        nc.scalar.activation(
                            G2, gf, AF.Copy, scale=rtf[:, qk:qk + 1]
                        )
                        P = work.tile([128, 512], BF16, tag="P")
                        nc.vector.tensor_mul(P, R, G2)
                        if pend[0] is not None:
                            emit_acc_ones(*pend[0])
                        pend[0] = (R, G2, P, qk, h)
                emit_acc_ones(*pend[0])
                out_t = work.tile([65, NS], F32, tag="uvw")
                for row in (0, 32, 64):
                    nc.scalar.activation(
                        out_t[row:row + 1, 0:512], accs[0][row:row + 1, :], AF.Copy
                    )
                    nc.scalar.activation(
                        out_t[row:row + 1, 512:1024], accs[1][row:row + 1, :], AF.Copy
                    )
                nc.sync.dma_start(out=uvw[lb, hidx], in_=out_t[0:65:32, :])

            for lb in range(BPC):
                wt = {}
                F0 = C0 = None
                rtf0 = rtc0 = None
                for side in range(4):
                    cs = 0 if side == 0 else 1
                    if cs not in wt:
                        w_t = io.tile([KP, KC, NS], BF16, tag="wt")
                        nc.sync.dma_start(
                            out=w_t,
                            in_=wmat[lb, cs].rearrange("(k p) n -> p k n", p=KP),
                        )
                        wt[cs] = w_t
                    img_t = io.tile([KP, KC, CF], BF16, tag="imf")
                    nc.sync.dma_start(
                        out=img_t,
                        in_=imgf[lb, side].rearrange("(k p) c -> p k c", p=KP),
                    )
                    tagF = "F0" if side == 0 else "FS"
                    Fs, faccs = sample(img_t, MF, wt[cs], tagF)
                    rtf = norms(faccs, lb, side, 0)
                    imgc_t = io.tile([KP, KC, CC], BF16, tag="imc")
                    nc.sync.dma_start(
                        out=imgc_t,
                        in_=imgc[lb, side].rearrange("(k p) c -> p k c", p=KP),
                    )
                    tagC = "C0" if side == 0 else "CS"
                    Cs, caccs = sample(imgc_t, MC, wt[cs], tagC)
                    rtc = norms(caccs, lb, side, 1)
                    if side == 0:
                        F0, C0, rtf0, rtc0 = Fs, Cs, rtf, rtc
                    helper(F0, C0, Fs, Cs, rtf, rtc, lb, side)
    nc.finalize()
    return nc


def _build_W(coords):
    """Bilinear grid-sample weights [B, 784, 1024] fp32.

    Replicates the reference's fp32 coordinate math exactly; sample
    s = i*S + j reads coords[b, j, i] (grid = coords.transpose(0,2,1,3)).
    """
    g = coords.astype(np.float32) * np.float32(2.0) - np.float32(1.0)
    x = (g[..., 0] + np.float32(1.0)) * np.float32(0.5) * np.float32(W - 1)
    y = (g[..., 1] + np.float32(1.0)) * np.float32(0.5) * np.float32(H - 1)
    x0 = np.floor(x)
    y0 = np.floor(y)
    wx = x - x0
    wy = y - y0
    x0c = np.clip(x0, 0, W - 1).astype(np.int64)
    x1c = np.clip(x0 + 1, 0, W - 1).astype(np.int64)
    y0c = np.clip(y0, 0, H - 1).astype(np.int64)
    y1c = np.clip(y0 + 1, 0, H - 1).astype(np.int64)
    Wm = np.zeros((B, PX, NS), np.float32)
    bidx = np.arange(B)[:, None, None]
    jj, ii = np.meshgrid(np.arange(S), np.arange(S), indexing="ij")
    s = np.broadcast_to((ii * S + jj)[None], (B, S, S))
    one = np.float32(1.0)
    for px, wgt in (
        (y0c * W + x0c, (one - wx) * (one - wy)),
        (y1c * W + x0c, (one - wx) * wy),
        (y0c * W + x1c, wx * (one - wy)),
        (y1c * W + x1c, wx * wy),
    ):
        np.add.at(Wm, (bidx, px, s), wgt.astype(np.float32))
    return Wm


_NC_CACHE = None


def _get_nc():
    global _NC_CACHE
    if _NC_CACHE is None:
        _NC_CACHE = build_bass()
    return _NC_CACHE


def _run_sim(nc, in_maps):
    from concourse.bass_interp import CoreSim

    results = []
    for in_map in in_maps:
        sim = CoreSim(nc)
        for name, val in in_map.items():
            sim.tensor(name)[:] = val
        sim.simulate()
        results.append({
            "uvw": sim.tensor("uvw").copy(),
            "rn": sim.tensor("rn").copy(),
        })
    return results


def kernel(**inputs):
    of = np.asarray(inputs["orig_feats"], dtype=np.float32)
    ofp = np.asarray(inputs["orig_feats_pos"], dtype=np.float32)
    oc = np.asarray(inputs["orig_code"], dtype=np.float32)
    ocp = np.asarray(inputs["orig_code_pos"], dtype=np.float32)
    coords1 = np.asarray(inputs["coords1"], dtype=np.float32)
    coords2 = np.asarray(inputs["coords2"], dtype=np.float32)
    perms = np.asarray(inputs["perms"])

    W1 = _build_W(coords1)
    W2 = _build_W(coords2)
    bf = ml_dtypes.bfloat16
    ofT = np.ascontiguousarray(of.reshape(B, CF, PX).transpose(0, 2, 1))
    ofpT = np.ascontiguousarray(ofp.reshape(B, CF, PX).transpose(0, 2, 1))
    ocT = np.ascontiguousarray(oc.reshape(B, CC, PX).transpose(0, 2, 1))
    ocpT = np.ascontiguousarray(ocp.reshape(B, CC, PX).transpose(0, 2, 1))

    in_maps = []
    for core in range(NCORES):
        bs = [core * BPC + i for i in range(BPC)]
        imgf = np.stack(
            [np.stack([ofT[b], ofpT[b], ofT[perms[0, b]], ofT[perms[1, b]]]) for b in bs]
        )
        imgc = np.stack(
            [np.stack([ocT[b], ocpT[b], ocT[perms[0, b]], ocT[perms[1, b]]]) for b in bs]
        )
        wm = np.stack([np.stack([W1[b], W2[b]]) for b in bs])
        in_maps.append(
            {
                "imgf": imgf.astype(bf),
                "imgc": imgc.astype(bf),
                "wmat": wm.astype(bf),
            }
        )

    global LAST_IN_MAPS
    LAST_IN_MAPS = in_maps
    nc = _get_nc()
    if os.environ.get("BASS_KERNEL_SIM"):
        results = _run_sim(nc, in_maps)
    else:
        from concourse.bass_utils import run_bass_kernel_spmd

        results = run_bass_kernel_spmd(nc, in_maps, list(range(NCORES))).results

    acc = {h: [0.0, 0.0, 0.0, 0.0] for h in range(4)}  # S1, S2, S3, F
    for core in range(NCORES):
        uvw = np.asarray(results[core]["uvw"], dtype=np.float64)
        rn = np.asarray(results[core]["rn"], dtype=np.float64)
        for lb in range(BPC):
            rA = rn[lb, 0, 0].T.ravel()
            rC1 = rn[lb, 0, 1].T.ravel()
            for h in range(4):
                u1raw, v1raw, w1raw = uvw[lb, h]
                u1 = rC1 * u1raw
                v1 = rA * v1raw
                acc[h][0] += float(np.sum(rC1 * rA * w1raw))
                acc[h][1] += float(np.sum(u1 * v1)) / NS
                acc[h][2] += float(np.sum(u1))
                acc[h][3] += float(np.sum(rA * v1raw))

    losses = []
    for h in range(4):
        S1, S2, S3, F = acc[h]
        am = F / (B * NS * NS)
        losses.append(-(S1 - S2 + (am - SHIFTS[h]) * S3) / (B * NS * NS))
    return np.array(
        [losses[0], losses[1], 0.5 * (losses[2] + losses[3])], dtype=np.float32
    )
